# revision 19
# baseline (speedup 1.0000x reference)
"""Adaptive softmax (head + 2 projected tails) CE loss on 8 TRN2 NeuronCores.

Strategy: data parallelism over tokens (4096 tokens -> 512/core, no
collectives), weights replicated. Tail segments are *packed*: the host
gathers each core's tail0/tail1 tokens (~171 of 512 each, binomial; 256
capacity) into 2 dense 128-token tiles per tail, so tail matmul/exp/sum
work drops ~2x vs computing all 512 tokens densely and masking. This is
the actual adaptive-softmax algorithm (cluster gather); the reference
only computes densely because jax needs static shapes, and its masked
entries are exactly 0 in the output.

Per core:
  - head logits  x @ Wh        [512,1024]@[1024,20002]  fp8 DoubleRow matmul
  - tail0 logits (x0@P0) @ W0  via h0T = P0^T x0^T (fp8), packed 256 tokens
  - tail1 logits (x1@P1) @ W1  (K=64, plain fp8 matmul), packed 256 tokens
  - logsumexp per token per tile-row: exp on the scalar engine (fp8 scale
    factors folded into the activation's free pre-scale); row-sums fused
    via activation accum_out (head) or offloaded to the vector engine
    (tails), accumulated across 1024-wide vocab units
  - picked logit per token: host gathers the label's weight column (index
    prep only); device computes dot(x_n, W[:, lab_n]) in bf16 on the DVE
  - loss = ln(sumexp) - picked; host scatters packed tail losses back
fp8 quantization only touches the logits feeding logsumexp (lse is highly
noise-tolerant); the picked-logit path stays bf16. PSUM accumulation is fp32.

Schedule notes (hard-won):
  - head/tail0/tail1 units interleaved per (supertile, tok-tile) so the PE
    always has dense work: idle gaps >~0.5us re-throttle the PE clock (HAM)
  - PSUM: 4 rotating [128,1024] slots; finish each 512-wide accumulation
    group before switching banks (bank cycling costs ~75ns/matmul)
  - weight slabs stream per k-chunk so matmuls start on the first chunk
  - picked-logit DVE work emitted mid-loop (s=2..4) so it fills DVE slack
    instead of serializing at the end
"""

import sys

for _p in ("/opt/trn_rl_repo",):
    if _p not in sys.path:
        sys.path.insert(0, _p)

import numpy as np
import ml_dtypes

BF16 = ml_dtypes.bfloat16
FP8 = ml_dtypes.float8_e4m3

# ---- problem constants (hardcoded per spec) ----
B, S, H = 8, 512, 1024
N = B * S                      # 4096 tokens
NCORES = 8
TOK = N // NCORES              # 512 tokens per core
TTH = TOK // 128               # 4 head token tiles
CTOK = 256                     # packed tail token capacity per core
TTC = CTOK // 128              # 2 packed tail token tiles
NROW = TTH + 2 * TTC           # 8 output tile-rows: 4 head, 2 t0, 2 t1
KH = H // 128                  # 8 contraction chunks for H
V_HEAD = 20002
V_TAIL = 20000
P0, K0 = 256, 2                # tail0 proj dim
P1 = 64                        # tail1 proj dim
CUT0, CUT1 = 20000, 40000
SUP = 2048                     # vocab supertile width (4 PSUM banks)
KH2 = H // 256                 # DoubleRow contraction chunks (256 rows each)
# fp8 scale factors (values scaled into e4m3 range; descaled in the exp)
SX, SW, SP, SH = 8.0, 64.0, 64.0, 2.0


def _supertiles(v):
    # first supertile is a single 1024 unit so the first weight slab DMA
    # (512KB instead of 2MB) comes off the startup critical path; the
    # remainder lands in a small last supertile, shrinking the end drain
    out = [(0, 1024)]
    base = 1024
    while base < v:
        w = min(SUP, v - base)
        out.append((base, w))
        base += w
    return out


SUPS_HEAD = _supertiles(V_HEAD)   # 1024 + 9x2048 + 546
SUPS_TAIL = _supertiles(V_TAIL)   # 1024 + 9x2048 + 544
NSUP = len(SUPS_HEAD)             # 11 (same count for tails)
assert len(SUPS_TAIL) == NSUP
# per-supertile unit counts (same for head and tails) and running base index
_N_UNITS = [max(1, (w + 1023) // 1024) for _, w in SUPS_HEAD]
assert _N_UNITS == [max(1, (w + 1023) // 1024) for _, w in SUPS_TAIL]
UNIT_BASE = [0]
for _n in _N_UNITS:
    UNIT_BASE.append(UNIT_BASE[-1] + _n)
NUNITS = UNIT_BASE[-1]            # 20 units per tile-row

_NC_CACHE = None


def _build_nc():
    import concourse.bass as bass
    import concourse.tile as tile
    from concourse import bacc, mybir

    f32 = mybir.dt.float32
    bf16 = mybir.dt.bfloat16
    f8 = mybir.dt.float8e4
    DR = mybir.MatmulPerfMode.DoubleRow
    Act = mybir.ActivationFunctionType
    Alu = mybir.AluOpType

    nc = bacc.Bacc("TRN2", target_bir_lowering=False, debug=False)

    # inputs (per-core shards / replicated weights)
    d_xT = nc.dram_tensor("xT", [H, TOK], f8, kind="ExternalInput")
    d_xT0 = nc.dram_tensor("xT0", [H, CTOK], f8, kind="ExternalInput")
    d_xT1 = nc.dram_tensor("xT1", [H, CTOK], f8, kind="ExternalInput")
    d_p0 = nc.dram_tensor("p0", [H, P0], f8, kind="ExternalInput")
    # p1 duplicated into two 64-col copies: one matmul then yields h1T
    # replicated across both partition halves, enabling row-group-packed
    # (concurrent) K=64 tail1 matmuls for the two token tiles.
    d_p1 = nc.dram_tensor("p1", [H, 2 * P1], f8, kind="ExternalInput")
    d_x = nc.dram_tensor("x", [TOK, H], bf16, kind="ExternalInput")
    d_wg = nc.dram_tensor("wg", [TOK, H], bf16, kind="ExternalInput")
    d_w0g = nc.dram_tensor("w0g", [CTOK, P0], bf16, kind="ExternalInput")
    d_w1g = nc.dram_tensor("w1g", [CTOK, P1], bf16, kind="ExternalInput")
    d_wh = nc.dram_tensor("wh", [H, V_HEAD], f8, kind="ExternalInput")
    d_w0 = nc.dram_tensor("w0", [P0, V_TAIL], f8, kind="ExternalInput")
    d_w1 = nc.dram_tensor("w1", [P1, V_TAIL], f8, kind="ExternalInput")
    # out[p, r]: tile-rows r: 0-3 head tile t, 4-5 t0 packed tile, 6-7 t1
    d_out = nc.dram_tensor("out", [128, NROW], f32, kind="ExternalOutput")

    with tile.TileContext(nc) as tc:
        with (
            tc.tile_pool(name="sing", bufs=1) as sing,
            tc.tile_pool(name="wst", bufs=2) as wst,
            tc.tile_pool(name="psum", bufs=4, space="PSUM") as psum,
            tc.tile_pool(name="scr", bufs=3) as scr,
        ):
            # ---- resident SBUF tensors ----
            p0_sb = sing.tile([128, KH2, 2, P0], f8)
            p1_sb = sing.tile([128, KH2, 2, 2 * P1], f8)
            xT0_sb = sing.tile([128, KH2, 2, CTOK], f8)
            xT1_sb = sing.tile([128, KH2, 2, CTOK], f8)
            xT_sb = sing.tile([128, KH2, 2, TOK], f8)
            # prep-critical inputs first; xT is issued inside the s-loop
            # right after supertile 0's weight chunks (it's needed at the
            # first head unit, after prep)
            nc.sync.dma_start(out=p0_sb[:, :, :, :], in_=d_p0.ap().rearrange("(c r p) q -> p c r q", p=128, r=2))
            nc.sync.dma_start(out=xT0_sb[:, :, :, :], in_=d_xT0.ap().rearrange("(c r p) t -> p c r t", p=128, r=2))
            nc.sync.dma_start(out=p1_sb[:, :, :, :], in_=d_p1.ap().rearrange("(c r p) q -> p c r q", p=128, r=2))
            nc.sync.dma_start(out=xT1_sb[:, :, :, :], in_=d_xT1.ap().rearrange("(c r p) t -> p c r t", p=128, r=2))

            x_sb = sing.tile([128, TTH, H], bf16)
            wg_sb = sing.tile([128, TTH, H], bf16)
            w0g_sb = sing.tile([128, TTC, P0], bf16)
            w1g_sb = sing.tile([128, TTC, P1], bf16)

            def picked_dmas():
                # issued after the first weight slab so they stay off the
                # startup critical path (DVE consumes them mid-kernel)
                nc.sync.dma_start(out=x_sb[:, :, :], in_=d_x.ap().rearrange("(t p) h -> p t h", p=128))
                nc.sync.dma_start(out=wg_sb[:, :, :], in_=d_wg.ap().rearrange("(t p) h -> p t h", p=128))
                nc.sync.dma_start(out=w0g_sb[:, :, :], in_=d_w0g.ap().rearrange("(t p) c -> p t c", p=128))
                nc.sync.dma_start(out=w1g_sb[:, :, :], in_=d_w1g.ap().rearrange("(t p) c -> p t c", p=128))

            h0T_sb = sing.tile([128, K0, CTOK], f8)   # h0^T * SH, DoubleRow lhsT
            h1T_sb = sing.tile([128, CTOK], f8)        # h1^T replicated in both halves
            h0_sb = sing.tile([128, TTC, P0], bf16)    # token-major, for picked
            h1_sb = sing.tile([128, TTC, P1], bf16)

            acc = sing.tile([128, NROW, NUNITS], f32)  # exp-sum partials
            picked = sing.tile([128, NROW], f32)
            sums = sing.tile([128, NROW], f32)
            lnS = sing.tile([128, NROW], f32)
            loss = sing.tile([128, NROW], f32)

            UW = 1024  # compute-unit width (2 PSUM banks; pool runs 4-deep)

            # ---- h0T = P0^T @ x0^T [256,256] ; h1T = P1^T @ x1^T [64,256] ----
            # (fp8 DoubleRow; rescaled to SH on the way to fp8 SBUF)
            for c2 in range(K0):
                pt = psum.tile([128, UW], f32, tag="pt")
                for c in range(KH2):
                    nc.tensor.matmul(
                        pt[:, 0:CTOK],
                        lhsT=p0_sb[:, c, :, c2 * 128:(c2 + 1) * 128],
                        rhs=xT0_sb[:, c, :, :],
                        start=(c == 0), stop=(c == KH2 - 1),
                        perf_mode=DR,
                    )
                nc.vector.tensor_scalar_mul(h0T_sb[:, c2, :], pt[:, 0:CTOK], SH / (SX * SP))
            pt = psum.tile([128, UW], f32, tag="pt")
            for c in range(KH2):
                nc.tensor.matmul(
                    pt[:, 0:CTOK],
                    lhsT=p1_sb[:, c, :, :],
                    rhs=xT1_sb[:, c, :, :],
                    start=(c == 0), stop=(c == KH2 - 1),
                    perf_mode=DR,
                )
            nc.vector.tensor_scalar_mul(h1T_sb[:, :], pt[:, 0:CTOK], SH / (SX * SP))

            # ---- token-major h0 [tok, 256] / h1 [tok, 64] for picked dots ----
            pt0 = psum.tile([128, UW], f32, tag="pt")
            for t in range(TTC):
                for c in range(KH2):
                    nc.tensor.matmul(
                        pt0[:, t * P0:(t + 1) * P0],
                        lhsT=xT0_sb[:, c, :, t * 128:(t + 1) * 128],
                        rhs=p0_sb[:, c, :, :],
                        start=(c == 0), stop=(c == KH2 - 1),
                        perf_mode=DR,
                    )
            pt1 = psum.tile([128, UW], f32, tag="pt")
            for t in range(TTC):
                for c in range(KH2):
                    nc.tensor.matmul(
                        pt1[:, t * P1:(t + 1) * P1],
                        lhsT=xT1_sb[:, c, :, t * 128:(t + 1) * 128],
                        rhs=p1_sb[:, c, :, 0:P1],
                        start=(c == 0), stop=(c == KH2 - 1),
                        perf_mode=DR,
                    )
            for t in range(TTC):
                nc.vector.tensor_scalar_mul(h0_sb[:, t, :], pt0[:, t * P0:(t + 1) * P0], 1.0 / (SX * SP))
                nc.vector.tensor_scalar_mul(h1_sb[:, t, :], pt1[:, t * P1:(t + 1) * P1], 1.0 / (SX * SP))

            # ---- main vocab loops: matmul unit -> fused exp+rowsum ----
            # Segments are interleaved per (unit, tok-tile) so the PE always
            # has dense head work between the small tail units (keeps the HAM
            # clock-gate warm); 4-deep PSUM rotation hides ACT drain latency.
            wh_r = d_wh.ap().rearrange("(c r p) v -> p c r v", p=128, r=2)
            w0_r = d_w0.ap().rearrange("(r p) v -> p r v", p=128)
            w1_r = d_w1.ap()

            ESC_H = 1.0 / (SX * SW)   # head exp descale
            ESC_T = 1.0 / (SH * SW)   # tail exp descale

            def unit(row, act_accum, t, sidx, ub, w, mm_emit, esc):
                pt = psum.tile([128, UW], f32, tag="pt")
                # n-outer / k-inner: finish each 512-slice accumulation group
                # before switching PSUM banks (bank cycling between
                # consecutive matmuls costs ~75ns/MM in micro-stalls)
                nb = 0
                while nb < w:
                    nw = min(512, w - nb)
                    mm_emit(pt, t, ub + nb, nb, nw)
                    nb += nw
                ex = scr.tile([128, UW], bf16, tag="exp")
                if act_accum:
                    # fused exp+row-sum on the scalar engine (pays the
                    # accumulator-drain READ on ACT)
                    nc.scalar.activation(
                        out=ex[:, 0:w], in_=pt[:, 0:w], func=Act.Exp, scale=esc,
                        accum_out=acc[:, row, sidx:sidx + 1],
                    )
                else:
                    # plain exp; row-sum offloaded to the (slack) DVE
                    nc.scalar.activation(
                        out=ex[:, 0:w], in_=pt[:, 0:w], func=Act.Exp, scale=esc,
                    )
                    nc.vector.tensor_reduce(
                        out=acc[:, row, sidx:sidx + 1], in_=ex[:, 0:w],
                        axis=mybir.AxisListType.X, op=Alu.add,
                    )

            def picked_head():
                for t in range(TTH):
                    sc = scr.tile([128, H], bf16, tag="ttr")
                    nc.vector.tensor_mul(sc[:, 0:H], x_sb[:, t, :], wg_sb[:, t, :])
                    nc.vector.tensor_reduce(
                        out=picked[:, t:t + 1], in_=sc[:, 0:H],
                        axis=mybir.AxisListType.X, op=Alu.add,
                    )

            def picked_t0():
                for t in range(TTC):
                    sc = scr.tile([128, H], bf16, tag="ttr")
                    nc.vector.tensor_mul(sc[:, 0:P0], h0_sb[:, t, :], w0g_sb[:, t, :])
                    nc.vector.tensor_reduce(
                        out=picked[:, TTH + t:TTH + t + 1], in_=sc[:, 0:P0],
                        axis=mybir.AxisListType.X, op=Alu.add,
                    )

            def picked_t1():
                for t in range(TTC):
                    sc = scr.tile([128, H], bf16, tag="ttr")
                    nc.vector.tensor_mul(sc[:, 0:P1], h1_sb[:, t, :], w1g_sb[:, t, :])
                    nc.vector.tensor_reduce(
                        out=picked[:, TTH + TTC + t:TTH + TTC + t + 1], in_=sc[:, 0:P1],
                        axis=mybir.AxisListType.X, op=Alu.add,
                    )

            for s in range(NSUP):
                hb, hw = SUPS_HEAD[s]
                tb, tw = SUPS_TAIL[s]
                # k-chunked slab DMAs: first matmuls start as soon as the
                # first chunk lands instead of waiting for the whole slab
                wt_h = wst.tile([128, KH2, 2, SUP], f8, tag="wh")
                for c in range(KH2):
                    nc.sync.dma_start(out=wt_h[:, c, :, 0:hw], in_=wh_r[:, c, :, hb:hb + hw])
                if s == 0:
                    nc.sync.dma_start(out=xT_sb[:, :, :, :], in_=d_xT.ap().rearrange("(c r p) t -> p c r t", p=128, r=2))
                wt_0 = wst.tile([128, 2, SUP], f8, tag="w0")
                nc.sync.dma_start(out=wt_0[:, :, 0:tw], in_=w0_r[:, :, tb:tb + tw])
                # w1 slab duplicated across both partition halves (row-pack)
                wt_1 = wst.tile([128, SUP], f8, tag="w1")
                nc.sync.dma_start(out=wt_1[0:P1, 0:tw], in_=w1_r[:, tb:tb + tw])
                nc.sync.dma_start(out=wt_1[P1:128, 0:tw], in_=w1_r[:, tb:tb + tw])
                if s == 0:
                    picked_dmas()

                def mm_head(pt, t, wb, nb, nw):
                    for c in range(KH2):
                        nc.tensor.matmul(
                            pt[:, nb:nb + nw],
                            lhsT=xT_sb[:, c, :, t * 128:(t + 1) * 128],
                            rhs=wt_h[:, c, :, wb:wb + nw],
                            start=(c == 0), stop=(c == KH2 - 1),
                            perf_mode=DR,
                        )

                def mm_t0(pt, t, wb, nb, nw):
                    nc.tensor.matmul(
                        pt[:, nb:nb + nw],
                        lhsT=h0T_sb[:, :, t * 128:(t + 1) * 128],
                        rhs=wt_0[:, :, wb:wb + nw],
                        start=True, stop=True,
                        perf_mode=DR,
                    )

                def mm_t1(pt, t, wb, nb, nw):
                    # tile t uses PE row-group half t (base partition 64*t):
                    # the two token tiles' K=64 matmuls execute concurrently
                    base = P1 * t
                    nc.tensor.matmul(
                        pt[:, nb:nb + nw],
                        lhsT=h1T_sb[base:base + P1, t * 128:(t + 1) * 128],
                        rhs=wt_1[base:base + P1, wb:wb + nw],
                        start=True, stop=True,
                    )

                for u in range(_N_UNITS[s]):  # unit-width chunks of the slab
                    ub = u * UW
                    uhw = min(UW, hw - ub)
                    utw = min(UW, tw - ub)
                    si = UNIT_BASE[s] + u
                    # head units interleaved with tail units so the PE never
                    # runs more than ~1 unit ahead of the ACT/DVE drain on
                    # the 4 PSUM slots; the two t1 units sit adjacent so
                    # their row-group-packed matmuls overlap in the array.
                    # Half the head units fuse their row-sum on ACT, half
                    # offload to DVE — balances ACT vs DVE budgets.
                    unit(0, u == 0, 0, si, ub, uhw, mm_head, ESC_H)
                    unit(TTH + 0, False, 0, si, ub, utw, mm_t0, ESC_T)
                    unit(1, u == 1, 1, si, ub, uhw, mm_head, ESC_H)
                    unit(TTH + TTC + 0, False, 0, si, ub, utw, mm_t1, ESC_T)
                    unit(TTH + TTC + 1, False, 1, si, ub, utw, mm_t1, ESC_T)
                    unit(2, u == 0, 2, si, ub, uhw, mm_head, ESC_H)
                    unit(TTH + 1, False, 1, si, ub, utw, mm_t0, ESC_T)
                    unit(3, u == 1, 3, si, ub, uhw, mm_head, ESC_H)

                # picked dots run in DVE slack mid-loop (DMAs are done by s=2)
                if s == 2:
                    picked_head()
                elif s == 3:
                    picked_t0()
                elif s == 4:
                    picked_t1()

            # ---- epilogue: loss = ln(sumexp) - picked ----
            # ln via float-bit log2 estimate + one Newton step through the
            # already-loaded Exp table (avoids the ~1.3us natural_log
            # ACT_TABLE_LOAD at the tail). y0 = ln2*(E+m) +/- 0.0298;
            # y1 = y0 + (S*e^-y0 - 1) = ln(S) + O(err^2) <= 3e-4 abs.
            for r in range(NROW):
                nc.vector.tensor_reduce(
                    out=sums[:, r:r + 1], in_=acc[:, r, :],
                    axis=mybir.AxisListType.X, op=Alu.add,
                )
            LN2 = 0.6931471805599453
            nc.vector.tensor_scalar(
                out=lnS[:, :], in0=sums[:, :].bitcast(mybir.dt.int32),
                scalar1=LN2 / (1 << 23), scalar2=-127.0 * LN2 + 0.0298,
                op0=Alu.mult, op1=Alu.add,
            )
            expm = sing.tile([128, NROW], f32)
            nc.scalar.activation(out=expm[:, :], in_=lnS[:, :], func=Act.Exp, scale=-1.0)
            nc.vector.tensor_mul(expm[:, :], sums[:, :], expm[:, :])   # e^(lnS-y0)
            nc.vector.tensor_add(lnS[:, :], lnS[:, :], expm[:, :])     # y0 + 1 + d + d^2/2
            nc.vector.tensor_sub(loss[:, :], lnS[:, :], picked[:, :])
            nc.vector.tensor_scalar(
                out=loss[:, :], in0=loss[:, :], scalar1=1.0, scalar2=-1.0,
                op0=Alu.mult, op1=Alu.add,
            )
            nc.sync.dma_start(out=d_out.ap()[:, :], in_=loss[:, :])

    nc.compile()
    return nc


def get_nc():
    global _NC_CACHE
    if _NC_CACHE is None:
        _NC_CACHE = _build_nc()
    return _NC_CACHE


def _prep_inputs(inputs, labels, head_weight, tail_proj_0, tail_w_0,
                 tail_proj_1, tail_w_1):
    """Host-side shard + index prep. Returns (in_maps, scatter_idx)."""
    x = np.asarray(inputs, np.float32).reshape(N, H)
    lab = np.asarray(labels).reshape(N).astype(np.int64)
    wh = np.asarray(head_weight, np.float32)
    p0 = np.asarray(tail_proj_0, np.float32)
    w0 = np.asarray(tail_w_0, np.float32)
    p1 = np.asarray(tail_proj_1, np.float32)
    w1 = np.asarray(tail_w_1, np.float32)

    head_lab = np.where(lab >= CUT1, CUT0 + 1, np.where(lab >= CUT0, CUT0, lab))
    wg_all = wh.T[head_lab]      # [N, H]

    # replicated weights (cast once; fp8 operands pre-scaled into e4m3 range)
    wh_b = np.ascontiguousarray(wh * SW, dtype=FP8)
    w0_b = np.ascontiguousarray(w0 * SW, dtype=FP8)
    w1_b = np.ascontiguousarray(w1 * SW, dtype=FP8)
    p0_b = np.ascontiguousarray(p0 * SP, dtype=FP8)
    p1_b = np.ascontiguousarray(np.concatenate([p1, p1], axis=1) * SP, dtype=FP8)
    w0T = w0.T                   # [V_TAIL, P0]
    w1T = w1.T

    in_maps = []
    scatter = []                 # per core: (idx0 local, idx1 local)
    for c in range(NCORES):
        sl = slice(c * TOK, (c + 1) * TOK)
        xc = x[sl]
        labc = lab[sl]
        idx0 = np.nonzero((labc >= CUT0) & (labc < CUT1))[0]
        idx1 = np.nonzero(labc >= CUT1)[0]
        if len(idx0) > CTOK or len(idx1) > CTOK:
            raise ValueError(
                f"core {c}: tail token count {len(idx0)}/{len(idx1)} exceeds "
                f"packed capacity {CTOK}")
        x0 = np.zeros((CTOK, H), np.float32)
        x0[:len(idx0)] = xc[idx0]
        x1 = np.zeros((CTOK, H), np.float32)
        x1[:len(idx1)] = xc[idx1]
        w0g = np.zeros((CTOK, P0), np.float32)
        w0g[:len(idx0)] = w0T[labc[idx0] - CUT0]
        w1g = np.zeros((CTOK, P1), np.float32)
        w1g[:len(idx1)] = w1T[labc[idx1] - CUT1]
        scatter.append((idx0, idx1))
        in_maps.append({
            "xT": np.ascontiguousarray(xc.T * SX, dtype=FP8),
            "xT0": np.ascontiguousarray(x0.T * SX, dtype=FP8),
            "xT1": np.ascontiguousarray(x1.T * SX, dtype=FP8),
            "x": np.ascontiguousarray(xc, dtype=BF16),
            "wg": np.ascontiguousarray(wg_all[sl], dtype=BF16),
            "w0g": np.ascontiguousarray(w0g, dtype=BF16),
            "w1g": np.ascontiguousarray(w1g, dtype=BF16),
            "wh": wh_b, "w0": w0_b, "w1": w1_b, "p0": p0_b, "p1": p1_b,
        })
    return in_maps, scatter


def _assemble(results, scatter):
    """results: list of 8 dicts with 'out' [128, NROW] -> full [3*N] f32."""
    full = np.zeros((3, N), np.float32)
    for c in range(NCORES):
        o = np.asarray(results[c]["out"], np.float32)  # [128, NROW]
        # head: tile-rows 0..TTH-1, token t*128+p at [p, t]
        full[2, c * TOK:(c + 1) * TOK] = o[:, 0:TTH].T.reshape(TOK)
        idx0, idx1 = scatter[c]
        t0 = o[:, TTH:TTH + TTC].T.reshape(CTOK)
        t1 = o[:, TTH + TTC:NROW].T.reshape(CTOK)
        full[0, c * TOK + idx0] = t0[:len(idx0)]
        full[1, c * TOK + idx1] = t1[:len(idx1)]
    return full.reshape(-1)


def kernel(inputs, labels, head_weight, tail_proj_0, tail_w_0,
           tail_proj_1, tail_w_1):
    from concourse.bass_utils import run_bass_kernel_spmd

    nc = get_nc()
    in_maps, scatter = _prep_inputs(inputs, labels, head_weight, tail_proj_0,
                                    tail_w_0, tail_proj_1, tail_w_1)
    res = run_bass_kernel_spmd(nc, in_maps, core_ids=list(range(NCORES)))
    return _assemble(res.results, scatter)


# revision 20
# speedup vs baseline: 1.0178x; 1.0178x over previous
"""Adaptive softmax (head + 2 projected tails) CE loss on 8 TRN2 NeuronCores.

Strategy: data parallelism over tokens (4096 tokens -> 512/core, no
collectives), weights replicated. Tail segments are *packed*: the host
gathers each core's tail0/tail1 tokens (~171 of 512 each, binomial; 256
capacity) into 2 dense 128-token tiles per tail, so tail matmul/exp/sum
work drops ~2x vs computing all 512 tokens densely and masking. This is
the actual adaptive-softmax algorithm (cluster gather); the reference
only computes densely because jax needs static shapes, and its masked
entries are exactly 0 in the output.

Per core:
  - head logits  x @ Wh        [512,1024]@[1024,20002]  fp8 DoubleRow matmul
  - tail0 logits (x0@P0) @ W0  via h0T = P0^T x0^T (fp8), packed 256 tokens
  - tail1 logits (x1@P1) @ W1  (K=64, plain fp8 matmul), packed 256 tokens
  - logsumexp per token per tile-row: exp on the scalar engine (fp8 scale
    factors folded into the activation's free pre-scale); row-sums fused
    via activation accum_out (head) or offloaded to the vector engine
    (tails), accumulated across 1024-wide vocab units
  - picked logit per token: host gathers the label's weight column (index
    prep only); device computes dot(x_n, W[:, lab_n]) in bf16 on the DVE
  - loss = ln(sumexp) - picked; host scatters packed tail losses back
fp8 quantization only touches the logits feeding logsumexp (lse is highly
noise-tolerant); the picked-logit path stays bf16. PSUM accumulation is fp32.

Schedule notes (hard-won):
  - head/tail0/tail1 units interleaved per (supertile, tok-tile) so the PE
    always has dense work: idle gaps >~0.5us re-throttle the PE clock (HAM)
  - PSUM: 4 rotating [128,1024] slots; finish each 512-wide accumulation
    group before switching banks (bank cycling costs ~75ns/matmul)
  - weight slabs stream per k-chunk so matmuls start on the first chunk
  - picked-logit DVE work emitted mid-loop (s=2..4) so it fills DVE slack
    instead of serializing at the end
"""

import sys

for _p in ("/opt/trn_rl_repo",):
    if _p not in sys.path:
        sys.path.insert(0, _p)

import numpy as np
import ml_dtypes

BF16 = ml_dtypes.bfloat16
FP8 = ml_dtypes.float8_e4m3

# ---- problem constants (hardcoded per spec) ----
B, S, H = 8, 512, 1024
N = B * S                      # 4096 tokens
NCORES = 8
TOK = N // NCORES              # 512 tokens per core
TTH = TOK // 128               # 4 head token tiles
CTOK = 256                     # packed tail token capacity per core
TTC = CTOK // 128              # 2 packed tail token tiles
NROW = TTH + 2 * TTC           # 8 output tile-rows: 4 head, 2 t0, 2 t1
KH = H // 128                  # 8 contraction chunks for H
V_HEAD = 20002
V_TAIL = 20000
P0, K0 = 256, 2                # tail0 proj dim
P1 = 64                        # tail1 proj dim
CUT0, CUT1 = 20000, 40000
SUP = 2048                     # vocab supertile width (4 PSUM banks)
KH2 = H // 256                 # DoubleRow contraction chunks (256 rows each)
# fp8 scale factors (values scaled into e4m3 range; descaled in the exp)
SX, SW, SP, SH = 8.0, 64.0, 64.0, 2.0


def _supertiles(v):
    out = []
    base = 0
    while base < v:
        w = min(SUP, v - base)
        out.append((base, w))
        base += w
    return out


SUPS_HEAD = _supertiles(V_HEAD)   # 9x2048 + 1570
SUPS_TAIL = _supertiles(V_TAIL)   # 9x2048 + 1568
NSUP = len(SUPS_HEAD)             # 10 (same count for tails)
assert len(SUPS_TAIL) == NSUP
# per-supertile unit counts (same for head and tails) and running base index
_N_UNITS = [max(1, (w + 1023) // 1024) for _, w in SUPS_HEAD]
assert _N_UNITS == [max(1, (w + 1023) // 1024) for _, w in SUPS_TAIL]
UNIT_BASE = [0]
for _n in _N_UNITS:
    UNIT_BASE.append(UNIT_BASE[-1] + _n)
NUNITS = UNIT_BASE[-1]            # 20 units per tile-row

_NC_CACHE = None


def _build_nc():
    import concourse.bass as bass
    import concourse.tile as tile
    from concourse import bacc, mybir

    f32 = mybir.dt.float32
    bf16 = mybir.dt.bfloat16
    f8 = mybir.dt.float8e4
    DR = mybir.MatmulPerfMode.DoubleRow
    Act = mybir.ActivationFunctionType
    Alu = mybir.AluOpType

    nc = bacc.Bacc("TRN2", target_bir_lowering=False, debug=False)

    # inputs (per-core shards / replicated weights)
    d_xT = nc.dram_tensor("xT", [H, TOK], f8, kind="ExternalInput")
    d_xT0 = nc.dram_tensor("xT0", [H, CTOK], f8, kind="ExternalInput")
    d_xT1 = nc.dram_tensor("xT1", [H, CTOK], f8, kind="ExternalInput")
    d_p0 = nc.dram_tensor("p0", [H, P0], f8, kind="ExternalInput")
    # p1 duplicated into two 64-col copies: one matmul then yields h1T
    # replicated across both partition halves, enabling row-group-packed
    # (concurrent) K=64 tail1 matmuls for the two token tiles.
    d_p1 = nc.dram_tensor("p1", [H, 2 * P1], f8, kind="ExternalInput")
    d_x = nc.dram_tensor("x", [TOK, H], bf16, kind="ExternalInput")
    d_wg = nc.dram_tensor("wg", [TOK, H], bf16, kind="ExternalInput")
    d_w0g = nc.dram_tensor("w0g", [CTOK, P0], bf16, kind="ExternalInput")
    d_w1g = nc.dram_tensor("w1g", [CTOK, P1], bf16, kind="ExternalInput")
    d_wh = nc.dram_tensor("wh", [H, V_HEAD], f8, kind="ExternalInput")
    d_w0 = nc.dram_tensor("w0", [P0, V_TAIL], f8, kind="ExternalInput")
    d_w1 = nc.dram_tensor("w1", [P1, V_TAIL], f8, kind="ExternalInput")
    # out[p, r]: tile-rows r: 0-3 head tile t, 4-5 t0 packed tile, 6-7 t1
    d_out = nc.dram_tensor("out", [128, NROW], f32, kind="ExternalOutput")

    with tile.TileContext(nc) as tc:
        with (
            tc.tile_pool(name="sing", bufs=1) as sing,
            tc.tile_pool(name="wst", bufs=2) as wst,
            tc.tile_pool(name="psum", bufs=4, space="PSUM") as psum,
            tc.tile_pool(name="scr", bufs=3) as scr,
        ):
            # ---- resident SBUF tensors ----
            p0_sb = sing.tile([128, KH2, 2, P0], f8)
            p1_sb = sing.tile([128, KH2, 2, 2 * P1], f8)
            xT0_sb = sing.tile([128, KH2, 2, CTOK], f8)
            xT1_sb = sing.tile([128, KH2, 2, CTOK], f8)
            xT_sb = sing.tile([128, KH2, 2, TOK], f8)
            # prep-critical inputs first; xT is issued inside the s-loop
            # right after supertile 0's weight chunks (it's needed at the
            # first head unit, after prep)
            nc.sync.dma_start(out=p0_sb[:, :, :, :], in_=d_p0.ap().rearrange("(c r p) q -> p c r q", p=128, r=2))
            nc.sync.dma_start(out=xT0_sb[:, :, :, :], in_=d_xT0.ap().rearrange("(c r p) t -> p c r t", p=128, r=2))
            nc.sync.dma_start(out=p1_sb[:, :, :, :], in_=d_p1.ap().rearrange("(c r p) q -> p c r q", p=128, r=2))
            nc.sync.dma_start(out=xT1_sb[:, :, :, :], in_=d_xT1.ap().rearrange("(c r p) t -> p c r t", p=128, r=2))

            x_sb = sing.tile([128, TTH, H], bf16)
            wg_sb = sing.tile([128, TTH, H], bf16)
            w0g_sb = sing.tile([128, TTC, P0], bf16)
            w1g_sb = sing.tile([128, TTC, P1], bf16)

            def picked_dmas():
                # issued after the first weight slab so they stay off the
                # startup critical path (DVE consumes them mid-kernel)
                nc.sync.dma_start(out=x_sb[:, :, :], in_=d_x.ap().rearrange("(t p) h -> p t h", p=128))
                nc.sync.dma_start(out=wg_sb[:, :, :], in_=d_wg.ap().rearrange("(t p) h -> p t h", p=128))
                nc.sync.dma_start(out=w0g_sb[:, :, :], in_=d_w0g.ap().rearrange("(t p) c -> p t c", p=128))
                nc.sync.dma_start(out=w1g_sb[:, :, :], in_=d_w1g.ap().rearrange("(t p) c -> p t c", p=128))

            h0T_sb = sing.tile([128, K0, CTOK], f8)   # h0^T * SH, DoubleRow lhsT
            h1T_sb = sing.tile([128, CTOK], f8)        # h1^T replicated in both halves
            h0_sb = sing.tile([128, TTC, P0], bf16)    # token-major, for picked
            h1_sb = sing.tile([128, TTC, P1], bf16)

            acc = sing.tile([128, NROW, NUNITS], f32)  # exp-sum partials
            picked = sing.tile([128, NROW], f32)
            sums = sing.tile([128, NROW], f32)
            lnS = sing.tile([128, NROW], f32)
            loss = sing.tile([128, NROW], f32)

            UW = 1024  # compute-unit width (2 PSUM banks; pool runs 4-deep)

            # ---- h0T = P0^T @ x0^T [256,256] ; h1T = P1^T @ x1^T [64,256] ----
            # (fp8 DoubleRow; rescaled to SH on the way to fp8 SBUF)
            for c2 in range(K0):
                pt = psum.tile([128, UW], f32, tag="pt")
                for c in range(KH2):
                    nc.tensor.matmul(
                        pt[:, 0:CTOK],
                        lhsT=p0_sb[:, c, :, c2 * 128:(c2 + 1) * 128],
                        rhs=xT0_sb[:, c, :, :],
                        start=(c == 0), stop=(c == KH2 - 1),
                        perf_mode=DR,
                    )
                nc.vector.tensor_scalar_mul(h0T_sb[:, c2, :], pt[:, 0:CTOK], SH / (SX * SP))
            pt = psum.tile([128, UW], f32, tag="pt")
            for c in range(KH2):
                nc.tensor.matmul(
                    pt[:, 0:CTOK],
                    lhsT=p1_sb[:, c, :, :],
                    rhs=xT1_sb[:, c, :, :],
                    start=(c == 0), stop=(c == KH2 - 1),
                    perf_mode=DR,
                )
            nc.vector.tensor_scalar_mul(h1T_sb[:, :], pt[:, 0:CTOK], SH / (SX * SP))

            # ---- token-major h0 [tok, 256] / h1 [tok, 64] for picked dots ----
            pt0 = psum.tile([128, UW], f32, tag="pt")
            for t in range(TTC):
                for c in range(KH2):
                    nc.tensor.matmul(
                        pt0[:, t * P0:(t + 1) * P0],
                        lhsT=xT0_sb[:, c, :, t * 128:(t + 1) * 128],
                        rhs=p0_sb[:, c, :, :],
                        start=(c == 0), stop=(c == KH2 - 1),
                        perf_mode=DR,
                    )
            pt1 = psum.tile([128, UW], f32, tag="pt")
            for t in range(TTC):
                for c in range(KH2):
                    nc.tensor.matmul(
                        pt1[:, t * P1:(t + 1) * P1],
                        lhsT=xT1_sb[:, c, :, t * 128:(t + 1) * 128],
                        rhs=p1_sb[:, c, :, 0:P1],
                        start=(c == 0), stop=(c == KH2 - 1),
                        perf_mode=DR,
                    )
            for t in range(TTC):
                nc.vector.tensor_scalar_mul(h0_sb[:, t, :], pt0[:, t * P0:(t + 1) * P0], 1.0 / (SX * SP))
                nc.vector.tensor_scalar_mul(h1_sb[:, t, :], pt1[:, t * P1:(t + 1) * P1], 1.0 / (SX * SP))

            # ---- main vocab loops: matmul unit -> fused exp+rowsum ----
            # Segments are interleaved per (unit, tok-tile) so the PE always
            # has dense head work between the small tail units (keeps the HAM
            # clock-gate warm); 4-deep PSUM rotation hides ACT drain latency.
            wh_r = d_wh.ap().rearrange("(c r p) v -> p c r v", p=128, r=2)
            w0_r = d_w0.ap().rearrange("(r p) v -> p r v", p=128)
            w1_r = d_w1.ap()

            ESC_H = 1.0 / (SX * SW)   # head exp descale
            ESC_T = 1.0 / (SH * SW)   # tail exp descale

            def unit(row, act_accum, t, sidx, ub, w, mm_emit, esc):
                pt = psum.tile([128, UW], f32, tag="pt")
                # n-outer / k-inner: finish each 512-slice accumulation group
                # before switching PSUM banks (bank cycling between
                # consecutive matmuls costs ~75ns/MM in micro-stalls)
                nb = 0
                while nb < w:
                    nw = min(512, w - nb)
                    mm_emit(pt, t, ub + nb, nb, nw)
                    nb += nw
                ex = scr.tile([128, UW], bf16, tag="exp")
                if act_accum:
                    # fused exp+row-sum on the scalar engine (pays the
                    # accumulator-drain READ on ACT)
                    nc.scalar.activation(
                        out=ex[:, 0:w], in_=pt[:, 0:w], func=Act.Exp, scale=esc,
                        accum_out=acc[:, row, sidx:sidx + 1],
                    )
                else:
                    # plain exp; row-sum offloaded to the (slack) DVE
                    nc.scalar.activation(
                        out=ex[:, 0:w], in_=pt[:, 0:w], func=Act.Exp, scale=esc,
                    )
                    nc.vector.tensor_reduce(
                        out=acc[:, row, sidx:sidx + 1], in_=ex[:, 0:w],
                        axis=mybir.AxisListType.X, op=Alu.add,
                    )

            def picked_head():
                for t in range(TTH):
                    sc = scr.tile([128, H], bf16, tag="ttr")
                    nc.vector.tensor_mul(sc[:, 0:H], x_sb[:, t, :], wg_sb[:, t, :])
                    nc.vector.tensor_reduce(
                        out=picked[:, t:t + 1], in_=sc[:, 0:H],
                        axis=mybir.AxisListType.X, op=Alu.add,
                    )

            def picked_t0():
                for t in range(TTC):
                    sc = scr.tile([128, H], bf16, tag="ttr")
                    nc.vector.tensor_mul(sc[:, 0:P0], h0_sb[:, t, :], w0g_sb[:, t, :])
                    nc.vector.tensor_reduce(
                        out=picked[:, TTH + t:TTH + t + 1], in_=sc[:, 0:P0],
                        axis=mybir.AxisListType.X, op=Alu.add,
                    )

            def picked_t1():
                for t in range(TTC):
                    sc = scr.tile([128, H], bf16, tag="ttr")
                    nc.vector.tensor_mul(sc[:, 0:P1], h1_sb[:, t, :], w1g_sb[:, t, :])
                    nc.vector.tensor_reduce(
                        out=picked[:, TTH + TTC + t:TTH + TTC + t + 1], in_=sc[:, 0:P1],
                        axis=mybir.AxisListType.X, op=Alu.add,
                    )

            for s in range(NSUP):
                hb, hw = SUPS_HEAD[s]
                tb, tw = SUPS_TAIL[s]
                # k-chunked slab DMAs: first matmuls start as soon as the
                # first chunk lands instead of waiting for the whole slab
                wt_h = wst.tile([128, KH2, 2, SUP], f8, tag="wh")
                for c in range(KH2):
                    nc.sync.dma_start(out=wt_h[:, c, :, 0:hw], in_=wh_r[:, c, :, hb:hb + hw])
                if s == 0:
                    nc.sync.dma_start(out=xT_sb[:, :, :, :], in_=d_xT.ap().rearrange("(c r p) t -> p c r t", p=128, r=2))
                wt_0 = wst.tile([128, 2, SUP], f8, tag="w0")
                nc.sync.dma_start(out=wt_0[:, :, 0:tw], in_=w0_r[:, :, tb:tb + tw])
                # w1 slab duplicated across both partition halves (row-pack)
                wt_1 = wst.tile([128, SUP], f8, tag="w1")
                nc.sync.dma_start(out=wt_1[0:P1, 0:tw], in_=w1_r[:, tb:tb + tw])
                nc.sync.dma_start(out=wt_1[P1:128, 0:tw], in_=w1_r[:, tb:tb + tw])
                if s == 0:
                    picked_dmas()

                def mm_head(pt, t, wb, nb, nw):
                    for c in range(KH2):
                        nc.tensor.matmul(
                            pt[:, nb:nb + nw],
                            lhsT=xT_sb[:, c, :, t * 128:(t + 1) * 128],
                            rhs=wt_h[:, c, :, wb:wb + nw],
                            start=(c == 0), stop=(c == KH2 - 1),
                            perf_mode=DR,
                        )

                def mm_t0(pt, t, wb, nb, nw):
                    nc.tensor.matmul(
                        pt[:, nb:nb + nw],
                        lhsT=h0T_sb[:, :, t * 128:(t + 1) * 128],
                        rhs=wt_0[:, :, wb:wb + nw],
                        start=True, stop=True,
                        perf_mode=DR,
                    )

                def mm_t1(pt, t, wb, nb, nw):
                    # tile t uses PE row-group half t (base partition 64*t):
                    # the two token tiles' K=64 matmuls execute concurrently
                    base = P1 * t
                    nc.tensor.matmul(
                        pt[:, nb:nb + nw],
                        lhsT=h1T_sb[base:base + P1, t * 128:(t + 1) * 128],
                        rhs=wt_1[base:base + P1, wb:wb + nw],
                        start=True, stop=True,
                    )

                for u in range(_N_UNITS[s]):  # unit-width chunks of the slab
                    ub = u * UW
                    uhw = min(UW, hw - ub)
                    utw = min(UW, tw - ub)
                    si = UNIT_BASE[s] + u
                    # head units interleaved with tail units so the PE never
                    # runs more than ~1 unit ahead of the ACT/DVE drain on
                    # the 4 PSUM slots; the two t1 units sit adjacent so
                    # their row-group-packed matmuls overlap in the array.
                    # Half the head units fuse their row-sum on ACT, half
                    # offload to DVE — balances ACT vs DVE budgets.
                    unit(0, u == 0, 0, si, ub, uhw, mm_head, ESC_H)
                    unit(TTH + 0, False, 0, si, ub, utw, mm_t0, ESC_T)
                    unit(1, u == 1, 1, si, ub, uhw, mm_head, ESC_H)
                    unit(TTH + TTC + 0, False, 0, si, ub, utw, mm_t1, ESC_T)
                    unit(TTH + TTC + 1, False, 1, si, ub, utw, mm_t1, ESC_T)
                    unit(2, u == 0, 2, si, ub, uhw, mm_head, ESC_H)
                    unit(TTH + 1, False, 1, si, ub, utw, mm_t0, ESC_T)
                    unit(3, u == 1, 3, si, ub, uhw, mm_head, ESC_H)

                # picked dots run in DVE slack mid-loop (DMAs are done by s=2)
                if s == 2:
                    picked_head()
                elif s == 3:
                    picked_t0()
                elif s == 4:
                    picked_t1()

            # ---- epilogue: loss = ln(sumexp) - picked ----
            # ln via float-bit log2 estimate + one Newton step through the
            # already-loaded Exp table (avoids the ~1.3us natural_log
            # ACT_TABLE_LOAD at the tail). y0 = ln2*(E+m) +/- 0.0298;
            # y1 = y0 + (S*e^-y0 - 1) = ln(S) + O(err^2) <= 3e-4 abs.
            for r in range(NROW):
                nc.vector.tensor_reduce(
                    out=sums[:, r:r + 1], in_=acc[:, r, :],
                    axis=mybir.AxisListType.X, op=Alu.add,
                )
            LN2 = 0.6931471805599453
            nc.vector.tensor_scalar(
                out=lnS[:, :], in0=sums[:, :].bitcast(mybir.dt.int32),
                scalar1=LN2 / (1 << 23), scalar2=-127.0 * LN2 + 0.0298,
                op0=Alu.mult, op1=Alu.add,
            )
            expm = sing.tile([128, NROW], f32)
            nc.scalar.activation(out=expm[:, :], in_=lnS[:, :], func=Act.Exp, scale=-1.0)
            nc.vector.tensor_mul(expm[:, :], sums[:, :], expm[:, :])   # e^(lnS-y0)
            nc.vector.tensor_add(lnS[:, :], lnS[:, :], expm[:, :])     # y0 + 1 + d + d^2/2
            nc.vector.tensor_sub(loss[:, :], lnS[:, :], picked[:, :])
            nc.vector.tensor_scalar(
                out=loss[:, :], in0=loss[:, :], scalar1=1.0, scalar2=-1.0,
                op0=Alu.mult, op1=Alu.add,
            )
            nc.sync.dma_start(out=d_out.ap()[:, :], in_=loss[:, :])

    nc.compile()
    return nc


def get_nc():
    global _NC_CACHE
    if _NC_CACHE is None:
        _NC_CACHE = _build_nc()
    return _NC_CACHE


def _prep_inputs(inputs, labels, head_weight, tail_proj_0, tail_w_0,
                 tail_proj_1, tail_w_1):
    """Host-side shard + index prep. Returns (in_maps, scatter_idx)."""
    x = np.asarray(inputs, np.float32).reshape(N, H)
    lab = np.asarray(labels).reshape(N).astype(np.int64)
    wh = np.asarray(head_weight, np.float32)
    p0 = np.asarray(tail_proj_0, np.float32)
    w0 = np.asarray(tail_w_0, np.float32)
    p1 = np.asarray(tail_proj_1, np.float32)
    w1 = np.asarray(tail_w_1, np.float32)

    head_lab = np.where(lab >= CUT1, CUT0 + 1, np.where(lab >= CUT0, CUT0, lab))
    wg_all = wh.T[head_lab]      # [N, H]

    # replicated weights (cast once; fp8 operands pre-scaled into e4m3 range)
    wh_b = np.ascontiguousarray(wh * SW, dtype=FP8)
    w0_b = np.ascontiguousarray(w0 * SW, dtype=FP8)
    w1_b = np.ascontiguousarray(w1 * SW, dtype=FP8)
    p0_b = np.ascontiguousarray(p0 * SP, dtype=FP8)
    p1_b = np.ascontiguousarray(np.concatenate([p1, p1], axis=1) * SP, dtype=FP8)
    w0T = w0.T                   # [V_TAIL, P0]
    w1T = w1.T

    in_maps = []
    scatter = []                 # per core: (idx0 local, idx1 local)
    for c in range(NCORES):
        sl = slice(c * TOK, (c + 1) * TOK)
        xc = x[sl]
        labc = lab[sl]
        idx0 = np.nonzero((labc >= CUT0) & (labc < CUT1))[0]
        idx1 = np.nonzero(labc >= CUT1)[0]
        if len(idx0) > CTOK or len(idx1) > CTOK:
            raise ValueError(
                f"core {c}: tail token count {len(idx0)}/{len(idx1)} exceeds "
                f"packed capacity {CTOK}")
        x0 = np.zeros((CTOK, H), np.float32)
        x0[:len(idx0)] = xc[idx0]
        x1 = np.zeros((CTOK, H), np.float32)
        x1[:len(idx1)] = xc[idx1]
        w0g = np.zeros((CTOK, P0), np.float32)
        w0g[:len(idx0)] = w0T[labc[idx0] - CUT0]
        w1g = np.zeros((CTOK, P1), np.float32)
        w1g[:len(idx1)] = w1T[labc[idx1] - CUT1]
        scatter.append((idx0, idx1))
        in_maps.append({
            "xT": np.ascontiguousarray(xc.T * SX, dtype=FP8),
            "xT0": np.ascontiguousarray(x0.T * SX, dtype=FP8),
            "xT1": np.ascontiguousarray(x1.T * SX, dtype=FP8),
            "x": np.ascontiguousarray(xc, dtype=BF16),
            "wg": np.ascontiguousarray(wg_all[sl], dtype=BF16),
            "w0g": np.ascontiguousarray(w0g, dtype=BF16),
            "w1g": np.ascontiguousarray(w1g, dtype=BF16),
            "wh": wh_b, "w0": w0_b, "w1": w1_b, "p0": p0_b, "p1": p1_b,
        })
    return in_maps, scatter


def _assemble(results, scatter):
    """results: list of 8 dicts with 'out' [128, NROW] -> full [3*N] f32."""
    full = np.zeros((3, N), np.float32)
    for c in range(NCORES):
        o = np.asarray(results[c]["out"], np.float32)  # [128, NROW]
        # head: tile-rows 0..TTH-1, token t*128+p at [p, t]
        full[2, c * TOK:(c + 1) * TOK] = o[:, 0:TTH].T.reshape(TOK)
        idx0, idx1 = scatter[c]
        t0 = o[:, TTH:TTH + TTC].T.reshape(CTOK)
        t1 = o[:, TTH + TTC:NROW].T.reshape(CTOK)
        full[0, c * TOK + idx0] = t0[:len(idx0)]
        full[1, c * TOK + idx1] = t1[:len(idx1)]
    return full.reshape(-1)


def kernel(inputs, labels, head_weight, tail_proj_0, tail_w_0,
           tail_proj_1, tail_w_1):
    from concourse.bass_utils import run_bass_kernel_spmd

    nc = get_nc()
    in_maps, scatter = _prep_inputs(inputs, labels, head_weight, tail_proj_0,
                                    tail_w_0, tail_proj_1, tail_w_1)
    res = run_bass_kernel_spmd(nc, in_maps, core_ids=list(range(NCORES)))
    return _assemble(res.results, scatter)
